# revision 20
# baseline (speedup 1.0000x reference)
"""2D DCT-II (4096x4096) on 8 Trainium2 NeuronCores (axon/PJRT SPMD).

Math: the reference computes C = AM @ y @ AN^T with y = x[pm][:, pn] (Makhoul
even-odd reorder); permutations fold into tables built from the expk inputs:
  AN[v,j] = 2*(cos(2pi*v*pinv[j]/N)*eNr[v] + sin(...)*eNi[v]),  AM similar w/ 0.5.
Both tables obey the DCT mirror symmetry  T[w, L-1-j] = (-1)^w T[w, j]  for ANY
expk (pinv[L-1-j] = pinv[j] + L/2 mod L), so x folds into 4 parity quadrants
  Xq[qu,qv][i',j'] = fold_rows(fold_cols(x))   (i',j' < 2048)
and C splits into 4 independent quarter problems (half the FLOPs per phase):
  C[2uh+qu, 2vh+qv] = AMq_qu @ Xq[qu,qv] @ ANq_qv^T.

Distribution (8 cores), all matmuls bf16 with fp32 PSUM accumulation.
Collective-free: the folded X ships to every core (cores start skewed by
~50us under PJRT dispatch; any collective would put that skew on the
critical path).  Core k owns output columns [512k, 512k+512):
  phase 1: W_q[i', vh_k] = Xq @ ANq^T[:, vh_k]   (X streamed in 1MB panels,
           W kept in SBUF bf16)
  phase 2: Cq^T[vh_k, uh] = W^T-slices @ AMq     (AM streamed once)
Host: folds x, transposes, casts bf16, pre-swizzles to partition-major
blocks so every DMA is >=8KB contiguous per partition; interleaves the
quarter outputs back to the full (4096, 4096) fp32.
"""
import numpy as np
import ml_dtypes

_NCORES = 8
_N = 4096
_H = 2048            # folded size
_VH = _H // _NCORES  # 256 output columns (vh) per core
_JT = _H // 128      # 16 contraction tiles
_state = {}


# --------------------------------------------------------------------------
# Bass kernel
# --------------------------------------------------------------------------
def _build_bass():
    import concourse.bacc as bacc
    import concourse.mybir as mybir
    from concourse.tile import TileContext

    bf16 = mybir.dt.bfloat16
    fp32 = mybir.dt.float32
    nc = bacc.Bacc("TRN2", target_bir_lowering=False, debug=False,
                   num_devices=_NCORES)
    # xqt[q, ib, p, (jt i)]: folded-transposed x, partition-major swizzled:
    #   element = Xq_q[i' = 256*ib + i_hi, j' = 128*jt + p]   (i block of 256)
    xqt = nc.declare_dram_parameter("xqt", [4, 8, 128, _JT * 256], bf16,
                                    isOutput=False)
    # antk[p, (qv jt v)]: this core's AN^T slice, partition-major:
    #   element = AN[2*(256k+v)+qv, j' = 128*jt + p]
    antk = nc.declare_dram_parameter("antk", [128, 2 * _JT * _VH], bf16,
                                     isOutput=False)
    # amt[qu, uc, p, (kt u)]: AM^T panels, partition-major:
    #   element = AM[2*(512*uc+u)+qu, i' = 128*kt + p]
    amt = nc.declare_dram_parameter("amt", [2, 4, 128, _JT * 512], bf16,
                                    isOutput=False)
    # cqt[q, vhl, uh] = C_q[uh, 256k+vhl]  (transposed quarter outputs)
    cqt = nc.declare_dram_parameter("cqt", [4, _VH, _H], bf16, isOutput=True)

    with TileContext(nc) as tc:
        with (
            tc.tile_pool(name="ank", bufs=1) as ank_pool,
            tc.tile_pool(name="xp", bufs=6) as xp_pool,
            tc.tile_pool(name="wsb", bufs=1) as wsb_pool,
            tc.tile_pool(name="ps1", bufs=4, space="PSUM") as ps1_pool,
            tc.tile_pool(name="am", bufs=4) as am_pool,
            tc.tile_pool(name="ps2", bufs=4, space="PSUM") as ps2_pool,
            tc.tile_pool(name="ev", bufs=6) as ev_pool,
        ):
            # ---------- phase 1: W_q[i', vh_k] = Xq @ ANq^T[:, vh_k] ------
            # the 2MB critical first loads (ank-qv0 + xp panel 0) get the
            # startup window exclusively; everything else is hinted later so
            # it doesn't share round-robin DMA bandwidth with them
            ank = ank_pool.tile([128, 2 * _JT * _VH], bf16)  # 16 KB/part
            half_an = _JT * _VH
            nc.sync.dma_start(out=ank[:, :half_an], in_=antk[:, :half_an])
            with tc.tile_wait_until(0.015):     # qv1 half first used ~56us in
                nc.sync.dma_start(out=ank[:, half_an:], in_=antk[:, half_an:])
            # W[p, (q kt v)] : W_q[i' = 128*kt + p, vh_k = v]
            wsb = wsb_pool.tile([128, 4 * _JT * _VH], bf16)  # 32 KB/part
            npanel = 0
            for q in (0, 2, 1, 3):      # phase-2 consumption order
                qv = q % 2
                for ib in range(8):
                    xp = xp_pool.tile([128, _JT * 256], bf16, tag="xp")
                    # panel n is consumed ~(14 + 3.5n)us in; stay ~4 ahead
                    hint = 0.0 if npanel == 0 else 0.006 + (npanel - 1) * 0.003
                    with tc.tile_wait_until(hint):
                        nc.sync.dma_start(out=xp[:], in_=xqt[q, ib])
                    npanel += 1
                    for half in range(2):
                        kt = 2 * ib + half
                        ps = ps1_pool.tile([128, _VH], fp32, tag="ps1")
                        for jt in range(_JT):
                            nc.tensor.matmul(
                                ps[:],
                                xp[:, jt * 256 + half * 128:
                                      jt * 256 + half * 128 + 128],
                                ank[:, (qv * _JT + jt) * _VH:
                                       (qv * _JT + jt + 1) * _VH],
                                start=(jt == 0), stop=(jt == _JT - 1))
                        nc.vector.tensor_copy(
                            wsb[:, (q * _JT + kt) * _VH:
                                   (q * _JT + kt + 1) * _VH], ps[:])

            # ---------- phase 2: Cq^T[vhl, uh] = W-slices.T @ AMq ---------
            for uc in range(4):
                ams = []
                for qu in range(2):
                    am = am_pool.tile([128, _JT * 512], bf16, tag="am")
                    # stream AM during phase 1, but out of the startup window
                    with tc.tile_wait_until(0.02 + uc * 0.025):
                        nc.sync.dma_start(out=am[:], in_=amt[qu, uc])  # 2MB
                    ams.append(am)
                for qv in range(2):
                    for qu in range(2):
                        q = 2 * qu + qv
                        am = ams[qu]
                        for vt in range(2):
                            ps = ps2_pool.tile([128, 512], fp32, tag="ps2")
                            for kt in range(_JT):
                                nc.tensor.matmul(
                                    ps[:],
                                    wsb[:, (q * _JT + kt) * _VH + vt * 128:
                                           (q * _JT + kt) * _VH + vt * 128
                                           + 128],
                                    am[:, kt * 512:(kt + 1) * 512],
                                    start=(kt == 0), stop=(kt == _JT - 1))
                            ev = ev_pool.tile([128, 512], bf16, tag="ev")
                            nc.vector.tensor_copy(ev[:], ps[:])
                            nc.sync.dma_start(
                                out=cqt[q, vt * 128:(vt + 1) * 128,
                                        uc * 512:(uc + 1) * 512],
                                in_=ev[:])

    nc.compile()
    return nc


# --------------------------------------------------------------------------
# PJRT SPMD runner (compile once, run many)
# --------------------------------------------------------------------------
def _build_runner(nc, n_cores):
    import jax
    from jax.sharding import Mesh, PartitionSpec
    from jax.experimental.shard_map import shard_map
    import concourse.mybir as mybir
    from concourse import bass2jax
    from concourse.bass2jax import _bass_exec_p, partition_id_tensor

    bass2jax.install_neuronx_cc_hook()
    partition_name = (nc.partition_id_tensor.name
                      if nc.partition_id_tensor else None)

    in_names, out_names, out_avals, zero_outs = [], [], [], []
    for alloc in nc.m.functions[0].allocations:
        if not isinstance(alloc, mybir.MemoryLocationSet):
            continue
        name = alloc.memorylocations[0].name
        if alloc.kind == "ExternalInput":
            if name != partition_name:
                in_names.append(name)
        elif alloc.kind == "ExternalOutput":
            shape = tuple(alloc.tensor_shape)
            dtype = mybir.dt.np(alloc.dtype)
            out_names.append(name)
            out_avals.append(jax.core.ShapedArray(shape, dtype))
            zero_outs.append(np.zeros(shape, dtype))
    n_params = len(in_names)
    n_outs = len(out_avals)
    in_names_all = list(in_names) + out_names
    if partition_name is not None:
        in_names_all = in_names_all + [partition_name]
    donate = tuple(range(n_params, n_params + n_outs))

    def _body(*args):
        operands = list(args)
        if partition_name is not None:
            operands.append(partition_id_tensor())
        outs = _bass_exec_p.bind(
            *operands,
            out_avals=tuple(out_avals),
            in_names=tuple(in_names_all),
            out_names=tuple(out_names),
            lowering_input_output_aliases=(),
            sim_require_finite=True,
            sim_require_nnan=True,
            nc=nc,
        )
        return tuple(outs)

    devices = jax.devices()[:n_cores]
    mesh = Mesh(np.asarray(devices), ("core",))
    smapped = shard_map(_body, mesh=mesh,
                        in_specs=(PartitionSpec("core"),) * (n_params + n_outs),
                        out_specs=(PartitionSpec("core"),) * n_outs,
                        check_rep=False)

    def dct_body(*args):
        return smapped(*args)

    sharded = jax.jit(dct_body, donate_argnums=donate, keep_unused=True)

    from jax.sharding import NamedSharding
    shard = NamedSharding(mesh, PartitionSpec("core"))
    _dev_cache = {}

    import jax.numpy as jnp
    _zero_shapes = [(n_cores * z.shape[0], *z.shape[1:]) for z in zero_outs]
    _zero_dtypes = [z.dtype for z in zero_outs]

    def dct_zeros():
        return tuple(jnp.zeros(s, d)
                     for s, d in zip(_zero_shapes, _zero_dtypes))

    _make_zeros = jax.jit(dct_zeros, out_shardings=(shard,) * len(_zero_shapes))

    def run(in_maps, cache_names=(), fetch=True):
        concat_in = []
        for i, name in enumerate(in_names):
            if name in cache_names and name in _dev_cache:
                concat_in.append(_dev_cache[name])
                continue
            arr = np.concatenate(
                [np.asarray(in_maps[c][name]) for c in range(n_cores)], axis=0)
            arr = jax.device_put(arr, shard)
            if name in cache_names:
                jax.block_until_ready(arr)
                _dev_cache[name] = arr
            concat_in.append(arr)
        concat_zeros = _make_zeros()
        raw = sharded(*concat_in, *concat_zeros)
        if not fetch:
            import jax as _jax
            _jax.block_until_ready(raw)
            return raw
        out_arrs = [np.asarray(o) for o in raw]
        return [
            {name: out_arrs[i].reshape(n_cores, *out_avals[i].shape)[c]
             for i, name in enumerate(out_names)}
            for c in range(n_cores)]

    run.dev_cache = _dev_cache
    return run


# --------------------------------------------------------------------------
# host-side tables
# --------------------------------------------------------------------------
def _tables(expkM, expkN):
    key = (expkM.tobytes(), expkN.tobytes())
    cached = _state.get("tables")
    if cached is not None and cached[0] == key:
        return cached[1], cached[2]
    run = _state.get("run")
    if run is not None:
        run.dev_cache.clear()
    n, h = _N, _H
    i = np.arange(n)
    pm = np.where(i < (n + 1) // 2, 2 * i, 2 * (n - i) - 1)
    pinv = np.empty(n, dtype=np.int64)
    pinv[pm] = i
    # only rows j' < 2048 of the full tables are needed after folding
    ang = (2.0 * np.pi / n) * np.outer(pinv[:h].astype(np.float64),
                                       i.astype(np.float64))
    Cp = np.cos(ang)
    Sp = np.sin(ang)
    eNr = expkN[:, 0].astype(np.float64)
    eNi = expkN[:, 1].astype(np.float64)
    eMr = expkM[:, 0].astype(np.float64)
    eMi = expkM[:, 1].astype(np.float64)
    annT = 2.0 * (Cp * eNr[None, :] + Sp * eNi[None, :])  # [j', v]
    amT = 0.5 * (Cp * eMr[None, :] + Sp * eMi[None, :])   # [i', u]
    bf16 = ml_dtypes.bfloat16
    # antk per core k: [128, (qv jt v)] partition-major
    #   = AN[2*(256k+v)+qv, 128*jt+p] = annT[128*jt+p, 2*(256k+v)+qv]
    ant = np.stack([annT[:, 0::2], annT[:, 1::2]])        # [qv, j', vh]
    ant = ant.reshape(2, _JT, 128, _NCORES, _VH).astype(bf16)
    antk = [np.ascontiguousarray(
        ant[:, :, :, k, :].transpose(2, 0, 1, 3).reshape(128, 2 * _JT * _VH))
        for k in range(_NCORES)]
    # amt: [qu, uc, 128, (kt u)] partition-major
    amq = np.stack([amT[:, 0::2], amT[:, 1::2]])          # [qu, i', uh]
    amq = amq.reshape(2, _JT, 128, 4, 512).astype(bf16)
    amts = np.ascontiguousarray(amq.transpose(0, 3, 2, 1, 4).reshape(
        2, 4, 128, _JT * 512))
    _state["tables"] = (key, antk, amts)
    return antk, amts


def _fold_x(x):
    """Fold x into 4 parity quadrants, transpose, cast bf16, and swizzle to
    [q, ib, p, (jt i)] partition-major blocks (8KB contiguous/partition)."""
    h = _H
    xt = x[:h]
    xb = x[h:][::-1]
    Xr = (xt + xb, xt - xb)
    Xq = np.empty((4, h, h), np.float32)
    for qu in range(2):
        L = Xr[qu][:, :h]
        R = Xr[qu][:, h:][:, ::-1]
        Xq[2 * qu] = L + R
        Xq[2 * qu + 1] = L - R
    # [q, i', j'] -> transposed [q, j', i'] -> blocks [q, ib, p, jt, i]
    xqt = Xq.transpose(0, 2, 1).astype(ml_dtypes.bfloat16)   # [q, j', i']
    xqt = xqt.reshape(4, _JT, 128, 8, 256)                   # [q,jt,p,ib,i]
    return np.ascontiguousarray(xqt.transpose(0, 3, 2, 1, 4).reshape(
        4, 8, 128, _JT * 256))


def _make_in_maps(x, expkM, expkN):
    antk, amts = _tables(expkM, expkN)
    xqtb = _fold_x(x)
    return [{"xqt": xqtb, "antk": antk[k], "amt": amts}
            for k in range(_NCORES)]


def kernel(x, expkM, expkN, M, N):
    x = np.asarray(x, dtype=np.float32)
    expkM = np.asarray(expkM, dtype=np.float32)
    expkN = np.asarray(expkN, dtype=np.float32)
    assert x.shape == (_N, _N) and int(M) == _N and int(N) == _N

    in_maps = _make_in_maps(x, expkM, expkN)
    if "run" not in _state:
        _state["run"] = _build_runner(_build_bass(), _NCORES)
    run = _state["run"]

    outs = run(in_maps, cache_names=("antk", "amt"))
    cq = np.stack([outs[k]["cqt"] for k in range(_NCORES)])
    # cq[k, 2qu+qv, vhl, uh] -> C[2uh+qu, 512k+2vhl+qv]
    C = (cq.astype(np.float32)
         .reshape(_NCORES, 2, 2, _VH, _H)
         .transpose(4, 1, 0, 3, 2)
         .reshape(_N, _N))
    return np.ascontiguousarray(C)
